# revision 5
# baseline (speedup 1.0000x reference)
"""4-layer LSTM (B=64, T=1024, F=256, H=512) on 8 Trainium2 NeuronCores.

Strategy: two 4-core layer pipelines, each handling half the batch (B=32).
Core c runs layer c%4 on batch half c//4. The sequence is processed in
chunks of C timesteps; each core ships its per-step transposed hidden
states (h.T, bf16) to the next core via one 4-rank AllGather per chunk
tick (groups [0-3] and [4-7]), with a 2-tick pipeline skew so the
collective overlaps compute.

With B=32 per core, the gate pre-activations for all four 512-wide gate
quarters pack into a single [128, 512] PSUM bank: partition 32q+b holds
batch element b of hidden-quarter q, with per-quarter column layout
[i f o g] (each 128 wide). Matmuls use 4x32-wide PE column tiling
(tile_position=(0,32q)) so all four quarters stream concurrently,
keeping the 128x128 PE array fully utilized at M=32. The recurrent
matmuls are split into two column phases (i,f then o,g) so the sigmoid
can start before the o/g columns finish accumulating. The combined bias
lands in PSUM via a K=4 one-hot matmul.

One PE transpose per step (bf16) yields all four hT K-chunks at once;
a DVE copy moves it to SBUF for the next step's stationary operand, and
an ACT copy writes the ship buffer (AllGather payload, which is also the
final output: the harness un-transposes on the host).

Core 0/4's AllGather slot carries garbage (core 3/7's h); one fused DVE
op per tick computes in_t = in_t*hmask + xpad (hmask=0 only there),
which simultaneously injects x into columns 64:128 (K-chunks 2,3, whose
W_ih rows are the real input weights; chunks 0,1 are zero rows).
"""

import sys

sys.path.insert(0, "/opt/trn_rl_repo")

import numpy as np
import ml_dtypes

import concourse.bass as bass
import concourse.mybir as mybir
from concourse import bacc
from concourse.tile import TileContext
from concourse.bass import ds
from concourse.masks import make_identity

BF16 = ml_dtypes.bfloat16

B, F, H, L = 64, 256, 512, 4
G = 4 * H  # 2048
NCORES = 8
SKEW = 2
T_FULL = 1024
C_DEFAULT = 16

_BUILD_CACHE = {}


def _gate_perm():
    """Permutation of the 4H gate columns.

    Quarter q (512 cols) = [i, f, o, g] each 128 wide, for hidden dims
    128q..128q+127. Original gate order in the reference is i, f, g, o.
    """
    perm = np.empty(G, dtype=np.int64)
    tmap = [0, 1, 3, 2]  # i, f, o, g -> original gate index
    d = np.arange(128)
    for q in range(4):
        for t in range(4):
            perm[512 * q + 128 * t + d] = tmap[t] * 512 + 128 * q + d
    return perm


def build(T, C, mode="real"):
    """Build and finalize the 8-core SPMD Bass program.

    mode="real": normal kernel. mode="sim": collectives replaced by local
    DMA (for TimelineSim).
    """
    NC_CH = T // C
    NTICKS = NC_CH + SKEW * (L - 1)
    f32 = mybir.dt.float32
    bf16 = mybir.dt.bfloat16

    nc = bacc.Bacc("TRN2", target_bir_lowering=False, debug=False,
                   num_devices=NCORES)

    w_in_T = nc.declare_dram_parameter("w_in_T", [128, 4, 4, 512], bf16,
                                       isOutput=False)
    w_hh_T = nc.declare_dram_parameter("w_hh_T", [128, 4, 4, 512], bf16,
                                       isOutput=False)
    bias4_d = nc.declare_dram_parameter("bias4", [4, 512], bf16,
                                        isOutput=False)
    onehot_d = nc.declare_dram_parameter("onehot", [4, 128], bf16,
                                         isOutput=False)
    kill_d = nc.declare_dram_parameter("kill", [128, NTICKS], f32,
                                       isOutput=False)
    hmask_d = nc.declare_dram_parameter("hmask", [128, 1], f32,
                                        isOutput=False)
    xT_d = nc.declare_dram_parameter("xT", [128, T, 128], bf16,
                                     isOutput=False)
    out_d = nc.declare_dram_parameter("out", [128, T, 128], bf16,
                                      isOutput=True)

    ship_shape = [128, C, 128]
    send_bufs = [nc.dram_tensor(f"send{p}", ship_shape, bf16)
                 for p in range(2)]
    # Local (non-shared) output: shared-output collectives need >4-core
    # groups; with 4-core groups each rank gets its own gathered copy.
    ag_outs = [nc.dram_tensor(f"agout{p}", [4 * 128, C, 128], bf16)
               for p in range(2)]

    with TileContext(nc) as tc:
        with (
            tc.tile_pool(name="const", bufs=1) as constp,
            tc.tile_pool(name="state", bufs=1) as statep,
            tc.tile_pool(name="inp", bufs=2) as inp,
            tc.tile_pool(name="stage", bufs=2) as stagep,
            tc.tile_pool(name="ew", bufs=3) as ewp,
            tc.tile_pool(name="gpsum", bufs=2, space="PSUM") as gpsum,
            tc.tile_pool(name="trpsum", bufs=2, space="PSUM") as trpsum,
        ):
            # ---- constants ----
            w_in_sb = constp.tile([128, 4, 4, 512], bf16)
            nc.sync.dma_start(out=w_in_sb[:], in_=w_in_T[:, :, :, :])
            w_hh_sb = constp.tile([128, 4, 4, 512], bf16)
            nc.sync.dma_start(out=w_hh_sb[:], in_=w_hh_T[:, :, :, :])
            bias_sb = constp.tile([4, 512], bf16)
            nc.sync.dma_start(out=bias_sb[:], in_=bias4_d[:, :])
            onehot_sb = constp.tile([4, 128], bf16)
            nc.sync.dma_start(out=onehot_sb[:], in_=onehot_d[:, :])
            kill_sb = constp.tile([128, NTICKS], f32)
            nc.sync.dma_start(out=kill_sb[:], in_=kill_d[:, :])
            hmask_sb = constp.tile([128, 1], f32)
            nc.sync.dma_start(out=hmask_sb[:], in_=hmask_d[:, :])
            ident = constp.tile([128, 128], bf16)
            make_identity(nc, ident[:])
            ident32 = constp.tile([128, 128], f32)
            make_identity(nc, ident32[:])

            # zero both send buffers (AG ticks 0 and 1 read pre-scan content)
            zt = constp.tile([128, C * 128], bf16)
            nc.vector.memset(zt[:], 0.0)
            for p in range(2):
                nc.sync.dma_start(out=send_bufs[p][:, :, :],
                                  in_=zt[:].rearrange("p (c k) -> p c k", c=C))

            # ---- persistent state ----
            c_state = statep.tile([128, 128], f32)
            nc.vector.memset(c_state[:], 0.0)
            hT_state = statep.tile([128, 128], bf16)
            nc.vector.memset(hT_state[:], 0.0)

            prev = (nc.gpsimd.partition_id() + 3) % 4

            Sigmoid = mybir.ActivationFunctionType.Sigmoid
            Tanh = mybir.ActivationFunctionType.Tanh
            Copy = mybir.ActivationFunctionType.Copy

            # carry = (so_bf, ship_tile, tick) of the previous tick's last
            # step, flushed at the top of the next tick
            carry = None

            def flush_h(so_sig, t1_t, t2_t, ship_tile, t, name, kill_col=None):
                """Produce step t's h.T = T(sigmoid(o)) * tanh(T(c)) into
                hT_state (critical chain) and the ship buffer (off-chain).
                Transposing sigma_o and c separately lets tanh run on the
                transposed tile, so h.T lands in SBUF with one DVE mul
                instead of h -> PE transpose -> PSUM -> copy. T(c) is built
                as T(t1) + T(t2) with two ACCUMULATING PE transposes, so the
                DVE add c=t1+t2 drops off the hT path (it still runs, but
                only feeds the next step's t1, which has slack). At tick
                boundaries the state kill is folded into the hT mul (the
                shipped copy stays unkilled: it is the producer's real h)."""
                # PE issue order [T(t1), trO, T(t2)] matches each input's
                # readiness (t1 < sigma_o < t2), so every transpose is
                # data-gated, not queue-gated behind its neighbor
                trC = trpsum.tile([128, 128], f32, tag="trC", name=f"C{name}")
                nc.tensor.matmul(trC[:], t1_t[:], ident32[:],
                                 is_transpose=True, start=True, stop=False)
                trO = trpsum.tile([128, 128], bf16, tag="trO", name=f"O{name}")
                nc.tensor.transpose(trO[:], so_sig[:], ident[:])
                trO_sb = ewp.tile([128, 128], bf16, tag="trOs")
                nc.vector.tensor_copy(trO_sb[:], trO[:])
                nc.tensor.matmul(trC[:], t2_t[:], ident32[:],
                                 is_transpose=True, start=False, stop=True)
                tcT = ewp.tile([128, 128], bf16, tag="tcT")
                nc.scalar.activation(tcT[:], trC[:], Tanh)
                if kill_col is None:
                    nc.vector.tensor_mul(hT_state[:], trO_sb[:], tcT[:])
                else:
                    nc.vector.scalar_tensor_tensor(
                        out=hT_state[:], in0=tcT[:], scalar=kill_col,
                        in1=trO_sb[:], op0=mybir.AluOpType.mult,
                        op1=mybir.AluOpType.mult)
                nc.vector.tensor_mul(ship_tile[:, t, :], trO_sb[:], tcT[:])

            def emit_ship_out(ship_tile, ptick):
                nc.sync.dma_start(out=send_bufs[ptick % 2][:, :, :],
                                  in_=ship_tile[:])
                ot = ptick - SKEW * (L - 1)
                if 0 <= ot < NC_CH:
                    nc.sync.dma_start(out=out_d[:, ot * C:(ot + 1) * C, :],
                                      in_=ship_tile[:])

            for tick in range(NTICKS):
                par = tick % 2
                # ---- collective: everyone ships its previous chunk ----
                if mode in ("sim", "probe"):
                    nc.gpsimd.dma_start(out=ag_outs[par][0:128, :, :],
                                        in_=send_bufs[par][:, :, :])
                else:
                    nc.gpsimd.collective_compute(
                        "AllGather", mybir.AluOpType.bypass,
                        replica_groups=[[0, 1, 2, 3], [4, 5, 6, 7]],
                        ins=[send_bufs[par].ap().opt()],
                        outs=[ag_outs[par].ap().opt()],
                    )
                # ---- IN fill: slot (rank-1 mod 4) of this AG ----
                in_t = inp.tile([128, C, 128], bf16, tag="IN")
                nc.gpsimd.dma_start(out=in_t[:],
                                    in_=ag_outs[par][ds(prev * 128, 128), :, :])
                if tick < NC_CH:
                    x_t = inp.tile([128, C, 128], bf16, tag="INX")
                    nc.sync.dma_start(out=x_t[:],
                                      in_=xT_d[:, tick * C:(tick + 1) * C, :])
                    # in_t = in_t*hmask + xpad: zeroes the garbage AG slot on
                    # cores 0/4 and injects x there; identity elsewhere.
                    # Split so step 0's in-proj only waits on the first slice.
                    for sl in (slice(0, 1), slice(1, C)):
                        nc.vector.scalar_tensor_tensor(
                            out=in_t[:, sl, :], in0=in_t[:, sl, :],
                            scalar=hmask_sb[:, 0:1], in1=x_t[:, sl, :],
                            op0=mybir.AluOpType.mult,
                            op1=mybir.AluOpType.add)

                ship_t = stagep.tile([128, C, 128], bf16, tag="SHIP")

                pending = None
                for t in range(C):
                    # Two separate PSUM banks: A = gate cols 0:256 (i,f),
                    # B = cols 256:512 (o,g). Separate tiles give the Tile
                    # framework region-level deps, so the sigmoid over (i,f)
                    # starts as soon as phase A's accumulation stops instead
                    # of waiting for the whole bank. (Also: exactly ONE
                    # start=True per PSUM bank — a second start clears the
                    # whole bank's has_written bits on HW.)
                    g_A = gpsum.tile([128, 512], f32, tag="gA",
                                     name=f"gA_{tick}_{t}")
                    g_B = gpsum.tile([128, 512], f32, tag="gB",
                                     name=f"gB_{tick}_{t}")
                    halves = ((g_A, 0, 256), (g_B, 256, 256))
                    # ---- bias (starts accumulation) + input projection;
                    # no h dependency: overlaps the previous step's chain ----
                    for g_ps, lo, w in halves:
                        nc.tensor.matmul(g_ps[:, 0:w], onehot_sb[:],
                                         bias_sb[:, lo:lo + w],
                                         start=True, stop=False)
                    # ---- previous step's transposes first: they gate the
                    # recurrent matmuls via hT, while the input projection
                    # below only gates the elementwise phase ----
                    if pending is not None:
                        po, pt1, pt2, pt = pending
                        flush_h(po, pt1, pt2, ship_t, pt, f"tr_{tick}_{pt}")
                    elif carry is not None:
                        po, pt1, pt2, pship, ptick = carry
                        flush_h(po, pt1, pt2, pship, C - 1, f"tr_{ptick}_c",
                                kill_col=kill_sb[:, tick:tick + 1])
                        emit_ship_out(pship, ptick)
                    for j in range(4):
                        if mode == "probe":
                            # timing probe: one full-M matmul per (j, half)
                            # costs the same as the span of 4 concurrent
                            # 32-wide col-tiled matmuls on real HW (data is
                            # wrong; TimelineSim never executes values)
                            for g_ps, lo, w in halves:
                                nc.tensor.matmul(
                                    g_ps[:, 0:w], in_t[:, t, 0:128],
                                    w_in_sb[:, j, 0, lo:lo + w],
                                    start=False, stop=False)
                            continue
                        lhsT = in_t[:, t, 32 * j:32 * j + 32]
                        for g_ps, lo, w in halves:
                            for q in range(4):
                                nc.tensor.matmul(
                                    g_ps[32 * q:32 * q + 32, 0:w], lhsT,
                                    w_in_sb[:, j, q, lo:lo + w],
                                    start=False, stop=False,
                                    tile_position=(0, 32 * q))
                    # ---- recurrent matmuls in two column phases:
                    # A = cols 0:256 (i,f), B = cols 256:512 (o,g).
                    # hT_state holds step t-1's h.T (DVE copy in flush_h) ----
                    for g_ps, lo, w in halves:
                        for j in range(4):
                            if mode == "probe":
                                # full-M, same engine time as the real
                                # 4-concurrent col-tiled span
                                nc.tensor.matmul(
                                    g_ps[:, 0:w], hT_state[:, 0:128],
                                    w_hh_sb[:, j, 0, lo:lo + w],
                                    start=False, stop=(j == 3))
                                continue
                            lhsT = hT_state[:, 32 * j:32 * j + 32]
                            for q in range(4):
                                nc.tensor.matmul(
                                    g_ps[32 * q:32 * q + 32, 0:w],
                                    lhsT, w_hh_sb[:, j, q, lo:lo + w],
                                    start=False, stop=(j == 3),
                                    tile_position=(0, 32 * q))
                    # ---- elementwise, straight from PSUM ----
                    sig = ewp.tile([128, 256], f32, tag="sig")
                    nc.scalar.activation(sig[:], g_A[:, 0:256], Sigmoid)
                    tg = ewp.tile([128, 128], f32, tag="tg")
                    nc.scalar.activation(tg[:], g_B[:, 128:256], Tanh)
                    so_bf = ewp.tile([128, 128], bf16, tag="sob")
                    nc.scalar.activation(so_bf[:], g_B[:, 0:128], Sigmoid)
                    t1 = ewp.tile([128, 128], f32, tag="t1")
                    if t == 0:
                        # fold the tick-boundary c kill into t1
                        nc.vector.scalar_tensor_tensor(
                            out=t1[:], in0=c_state[:],
                            scalar=kill_sb[:, tick:tick + 1],
                            in1=sig[:, 128:256], op0=mybir.AluOpType.mult,
                            op1=mybir.AluOpType.mult)
                    else:
                        nc.vector.tensor_mul(t1[:], sig[:, 128:256],
                                             c_state[:])
                    t2 = ewp.tile([128, 128], f32, tag="t2")
                    nc.vector.tensor_mul(t2[:], sig[:, 0:128], tg[:])
                    nc.vector.tensor_add(c_state[:], t1[:], t2[:])
                    pending = (so_bf, t1, t2, t) if t < C - 1 else None
                # last step's flush + this tick's send/out DMAs happen at the
                # top of the next tick (carried)
                carry = (so_bf, t1, t2, ship_t, tick)
            po, pt1, pt2, pship, ptick = carry
            flush_h(po, pt1, pt2, pship, C - 1, f"tr_{ptick}_c")
            emit_ship_out(pship, ptick)

    nc.finalize()
    return nc


def build_null(T, C):
    """Null program with identical external I/O — for timing calibration."""
    NC_CH = T // C
    NTICKS = NC_CH + SKEW * (L - 1)
    f32 = mybir.dt.float32
    bf16 = mybir.dt.bfloat16
    nc = bacc.Bacc("TRN2", target_bir_lowering=False, debug=False,
                   num_devices=NCORES)
    nc.declare_dram_parameter("w_in_T", [128, 4, 4, 512], bf16, isOutput=False)
    nc.declare_dram_parameter("w_hh_T", [128, 4, 4, 512], bf16, isOutput=False)
    nc.declare_dram_parameter("bias4", [4, 512], bf16, isOutput=False)
    nc.declare_dram_parameter("onehot", [4, 128], bf16, isOutput=False)
    kill_d = nc.declare_dram_parameter("kill", [128, NTICKS], f32,
                                       isOutput=False)
    nc.declare_dram_parameter("hmask", [128, 1], f32, isOutput=False)
    nc.declare_dram_parameter("xT", [128, T, 128], bf16, isOutput=False)
    out_d = nc.declare_dram_parameter("out", [128, T, 128], bf16,
                                      isOutput=True)
    with TileContext(nc) as tc:
        with tc.tile_pool(name="p", bufs=2) as pool:
            t = pool.tile([128, NTICKS], f32)
            nc.sync.dma_start(out=t[:, 0:NTICKS], in_=kill_d[:, :])
            tb = pool.tile([128, NTICKS], bf16)
            nc.vector.tensor_copy(tb[:], t[:])
            nc.sync.dma_start(out=out_d[:, 0:1, 0:NTICKS],
                              in_=tb[:, 0:NTICKS].rearrange(
                                  "p (a b) -> p a b", a=1))
    nc.finalize()
    return nc


def _prep_core_inputs(x_sh, weights, T, C):
    """Build the 8 per-core input maps from full (already shifted) inputs."""
    NC_CH = T // C
    NTICKS = NC_CH + SKEW * (L - 1)
    perm = _gate_perm()

    onehot = np.zeros((4, 128), np.float32)
    for q in range(4):
        onehot[q, 32 * q:32 * q + 32] = 1.0
    onehot = onehot.astype(BF16)
    zeros_xT = np.zeros([128, T, 128], dtype=BF16)

    in_maps = []
    for core in range(NCORES):
        l = core % 4
        half = core // 4
        W_ih, W_hh, b_ih, b_hh = weights[l]
        WiT = W_ih.T.astype(np.float32)  # [F_in, G]
        if WiT.shape[0] < 512:
            # layer 0: real rows at the x k-slots (256:512); the h k-slots
            # (0:256) are zero so the garbage AllGather slot is harmless
            WiT = np.concatenate(
                [np.zeros((512 - WiT.shape[0], G), np.float32), WiT], axis=0)
        WiT = WiT[:, perm]
        WhT = W_hh.T.astype(np.float32)[:, perm]
        bvec = (b_ih + b_hh).astype(np.float32)[perm]

        w_in = WiT.reshape(4, 128, 4, 512).transpose(1, 0, 2, 3).astype(BF16)
        w_hh = WhT.reshape(4, 128, 4, 512).transpose(1, 0, 2, 3).astype(BF16)
        bias4 = bvec.reshape(4, 512).astype(BF16)

        if l == 0:
            xh = x_sh[32 * half:32 * half + 32]  # [32, T, 256]
            xT = np.zeros((128, T, 128), np.float32)
            xT[:, :, 64:128] = (xh.transpose(2, 1, 0)      # [256, T, 32]
                                .reshape(2, 128, T, 32)
                                .transpose(1, 2, 0, 3)
                                .reshape(128, T, 64))
            xT = xT.astype(BF16)
            hmask = np.zeros((128, 1), np.float32)
        else:
            xT = zeros_xT
            hmask = np.ones((128, 1), np.float32)

        kill = np.ones((128, NTICKS), np.float32)
        kill[:, :min(SKEW * l + 1, NTICKS)] = 0.0

        in_maps.append({
            "w_in_T": w_in, "w_hh_T": w_hh, "bias4": bias4,
            "onehot": onehot, "xT": xT, "kill": kill, "hmask": hmask,
        })
    return in_maps


def _untranspose_out(arr, T):
    """[128, T, 128] hT dump -> [32, T, 512] batch-major h."""
    return (np.asarray(arr, dtype=np.float32)
            .reshape(128, T, 4, 32)
            .transpose(3, 1, 2, 0)
            .reshape(32, T, 512))


def run_lstm(x_sh, weights, T=T_FULL, C=C_DEFAULT):
    """x_sh: [B, T, F] float32 (already teacher-forcing shifted).
    weights: list of L tuples (W_ih, W_hh, b_ih, b_hh)."""
    import os
    from concourse import bass2jax
    key = (T, C)
    if key not in _BUILD_CACHE:
        _BUILD_CACHE[key] = build(T, C)
    nc = _BUILD_CACHE[key]
    in_maps = _prep_core_inputs(x_sh, weights, T, C)
    if os.environ.get("BASS_LSTM_TRACE", "0") == "1":
        from concourse import bass_utils
        res = bass_utils.run_bass_kernel_spmd(
            nc, in_maps, core_ids=list(range(NCORES)), trace=True,
            tmpdir="/tmp/lstm_trace",
            trace_cores=[int(os.environ.get("BASS_LSTM_TRACE_CORE", "3"))])
        print("exec_time_ns:", res.exec_time_ns)
        print("profile_json:", res.profile_json)
        results = res.results
    else:
        results = bass2jax.run_bass_via_pjrt(nc, in_maps, n_cores=NCORES)
    lo = _untranspose_out(results[L - 1]["out"], T)
    hi = _untranspose_out(results[2 * L - 1]["out"], T)
    return np.concatenate([lo, hi], axis=0)


def kernel(x, W_ih0, W_hh0, b_ih0, b_hh0, W_ih_rest, W_hh_rest, b_ih_rest,
           b_hh_rest, train_mode):
    x = np.asarray(x, dtype=np.float32)
    if int(train_mode):
        x_sh = np.concatenate(
            [np.zeros_like(x[:, :1]), x[:, :-1]], axis=1)
    else:
        x_sh = x
    weights = [(np.asarray(W_ih0, np.float32), np.asarray(W_hh0, np.float32),
                np.asarray(b_ih0, np.float32), np.asarray(b_hh0, np.float32))]
    for i in range(L - 1):
        weights.append((np.asarray(W_ih_rest[i], np.float32),
                        np.asarray(W_hh_rest[i], np.float32),
                        np.asarray(b_ih_rest[i], np.float32),
                        np.asarray(b_hh_rest[i], np.float32)))
    out = run_lstm(x_sh, weights, T=x.shape[1], C=C_DEFAULT)
    return np.asarray(out, dtype=np.float32)
